# revision 8
# baseline (speedup 1.0000x reference)
"""Trainium2 Bass kernel for nn_Attention_83803401879880 (BitNet-quantized causal attention).

Sharding: DP=4 over batch x TP=2 over heads (8 cores).
Core c: batch = c//2, head-group g = c%2 (heads 8g..8g+8 = channels 1024g..1024g+1024).

Math (must match reference exactly up to fp32 rounding):
  bitlinear: w_scale=max(mean|W|,1e-5); Wq=clip(rne(W/ws),-1,1)=sign(W)*[|W|>=0.5ws];
             a=max(absmax_row(x),1e-5); s=127/a; xq=rne(x*s) (clip vacuous);
             y = (xq @ Wq^T) * ws/s        [int matmul exact in bf16+fp32 psum]
  scores = floor(q@k^T * 45/256)  -> integers t
  causal mask -32767; m=rowmax; u=t-m
  e(u) = relu(u+24) + relu(10u+80) + relu(53u+159) - 7*[u>=-8]   (exact, integer u)
  probs = rne(e*255/sum(e));  out = clip(floor(probs@v/256),-128,127)
  y = bitlinear(out, Wo)

Engines: QKV/O projections bf16 (exact); scores & pv fp32 matmuls; softmax on
DVE/ACT with magic-constant rne (x+1.5*2^23-1.5*2^23) and floor(x)=rne(x-0.5).
"""
import sys

sys.path.insert(0, "/opt/trn_rl_repo")
sys.path.insert(0, "/root/.axon_site")

import numpy as np
from contextlib import ExitStack

import concourse.bass as bass
import concourse.mybir as mybir
import concourse.tile as tile
from concourse import bacc
from concourse.bass import ds
from concourse.bass_utils import run_bass_kernel_spmd
from concourse.masks import make_identity

F32 = mybir.dt.float32
BF16 = mybir.dt.bfloat16
AL = mybir.AluOpType
AF = mybir.ActivationFunctionType
AX = mybir.AxisListType

B, T, C = 4, 1024, 2048
H, D = 16, 128
HL = 8              # heads per core
CL = HL * D         # local channels = 1024
TB = T // 128       # 8 token blocks
CCH = C // 128      # 16 c-chunks
CLH = CL // 128     # 8 local c-chunks
MAGIC = float(np.float32(1.5 * 2 ** 23))
NEG = -32767.0
BIG = 1.0e9
RG2 = [[0, 1], [2, 3], [4, 5], [6, 7]]


def build_nc():
    nc = bacc.Bacc(None, target_bir_lowering=False)

    x_in = nc.dram_tensor("x", [T, C], F32, kind="ExternalInput")
    wq_in = nc.dram_tensor("wq", [CL, C], F32, kind="ExternalInput")
    wk_in = nc.dram_tensor("wk", [CL, C], F32, kind="ExternalInput")
    wv_in = nc.dram_tensor("wv", [CL, C], F32, kind="ExternalInput")
    wo_in = nc.dram_tensor("wo", [C, CL], F32, kind="ExternalInput")  # Wo[:, c_loc]
    y_out = nc.dram_tensor("y", [T // 2, C], F32, kind="ExternalOutput")

    with tile.TileContext(nc) as tc:
        with (
            tc.tile_pool(name="const", bufs=1) as constp,
            tc.tile_pool(name="oproj", bufs=1) as oprojp,
            tc.tile_pool(name="ps", bufs=2, space="PSUM") as psp,
            tc.tile_pool(name="psA", bufs=2, space="PSUM") as psAp,
            tc.tile_pool(name="psB", bufs=2, space="PSUM") as psBp,
            tc.tile_pool(name="dram", bufs=1, space="DRAM") as dramp,
        ):
            _cstack = ExitStack()
            wloadp = _cstack.enter_context(tc.tile_pool(name="wload", bufs=2))
            wternp = _cstack.enter_context(tc.tile_pool(name="wtern", bufs=2))
            wtp = _cstack.enter_context(tc.tile_pool(name="wt", bufs=2))
            xqtp = _cstack.enter_context(tc.tile_pool(name="xqt", bufs=1))
            qkvp = _cstack.enter_context(tc.tile_pool(name="qkv", bufs=2))
            attp = _cstack.enter_context(tc.tile_pool(name="att", bufs=2))
            smp = _cstack.enter_context(tc.tile_pool(name="sm", bufs=2))
            smvp = _cstack.enter_context(tc.tile_pool(name="smv", bufs=3))
            # ======== constants ========
            ones_col = constp.tile([128, 1], BF16)
            nc.vector.memset(ones_col[:], 1.0)
            ident = constp.tile([128, 128], F32)
            make_identity(nc, ident[:])
            # minmask: BIG on lower-incl-diag (key<=query), NEG above
            minmask = constp.tile([128, 128], F32)
            nc.gpsimd.memset(minmask[:], BIG)
            nc.gpsimd.affine_select(
                out=minmask[:], in_=minmask[:], compare_op=AL.is_ge,
                fill=NEG, base=0, pattern=[[-1, 128]], channel_multiplier=1)

            # ======== Phase A: w_scale partial sums ========
            ws_parts = constp.tile([1, 4], F32)
            for wi, (w_in, rows, cols) in enumerate(
                    [(wq_in, CL, C), (wk_in, CL, C), (wv_in, CL, C), (wo_in, C, CL)]):
                ps_ws = psAp.tile([1, 512], F32, tag="psA")
                mms = [(r0, c0) for r0 in range(0, rows, 128)
                       for c0 in range(0, cols, 512)]
                for r0 in range(0, rows, 128):
                    wchunk = wloadp.tile([128, cols], F32, tag="wchunk")
                    nc.sync.dma_start(wchunk[:], w_in[r0:r0 + 128, :])
                    wabs = wloadp.tile([128, cols], BF16, tag="wabs")
                    nc.scalar.activation(wabs[:], wchunk[:], AF.Abs)
                    for c0 in range(0, cols, 512):
                        i = mms.index((r0, c0))
                        nc.tensor.matmul(ps_ws[:], ones_col[:], wabs[:, c0:c0 + 512],
                                         start=(i == 0), stop=(i == len(mms) - 1))
                wsum = smvp.tile([1, 1], F32, tag="wsum")
                nc.vector.reduce_sum(wsum[:], ps_ws[:], axis=AX.X)
                nc.vector.tensor_copy(ws_parts[:, wi:wi + 1], wsum[:])

            ws_src = dramp.tile([1, 4], F32)
            ws_dst = dramp.tile([1, 4], F32)
            nc.sync.dma_start(ws_src[:], ws_parts[:])
            nc.gpsimd.collective_compute(
                "AllReduce", AL.add, replica_groups=RG2,
                ins=[ws_src.opt()], outs=[ws_dst.opt()])
            ws_row = constp.tile([1, 4], F32)
            nc.sync.dma_start(ws_row[:], ws_dst[:])
            nc.vector.tensor_scalar(ws_row[:], ws_row[:], 1.0 / (C * C), 1e-5,
                                    AL.mult, AL.max)
            ws_dram = dramp.tile([1, 4], F32)
            nc.sync.dma_start(ws_dram[:], ws_row[:])
            ws_bc = constp.tile([128, 4], F32)
            nc.sync.dma_start(ws_bc[:], ws_dram[:].partition_broadcast(128))
            thr_bc = constp.tile([128, 4], F32)
            nc.vector.tensor_scalar(thr_bc[:], ws_bc[:], 0.5, None, AL.mult)

            # ======== Phase X: x quantization ========
            a_col = constp.tile([128, TB], F32)
            s_col = constp.tile([128, TB], F32)
            xqT = [xqtp.tile([128, T], BF16, tag=f"xqT{cc}", name=f"xqT{cc}")
                   for cc in range(CCH)]
            for tb in range(TB):
                xblk = wloadp.tile([128, C], F32, tag="wchunk")
                nc.sync.dma_start(xblk[:], x_in[tb * 128:(tb + 1) * 128, :])
                nc.vector.tensor_reduce(a_col[:, tb:tb + 1], xblk[:], axis=AX.X,
                                        op=AL.max, apply_absolute_value=True)
                nc.vector.tensor_scalar(a_col[:, tb:tb + 1], a_col[:, tb:tb + 1],
                                        1e-5, None, AL.max)
                nc.vector.reciprocal(s_col[:, tb:tb + 1], a_col[:, tb:tb + 1])
                nc.vector.tensor_scalar(s_col[:, tb:tb + 1], s_col[:, tb:tb + 1],
                                        127.0, None, AL.mult)
                nc.vector.tensor_scalar(xblk[:], xblk[:], s_col[:, tb:tb + 1], MAGIC,
                                        AL.mult, AL.add)
                xqb = wternp.tile([128, C], BF16, tag="wmask", bufs=1)
                nc.vector.tensor_scalar(xqb[:], xblk[:], -MAGIC, None, AL.add)
                for cc in range(CCH):
                    nc.sync.dma_start_transpose(
                        xqT[cc][:, tb * 128:(tb + 1) * 128],
                        xqb[:, cc * 128:(cc + 1) * 128])

            # token-scale broadcast rows: a[token] along free dim
            aT_ps = psAp.tile([TB, 128], F32, tag="psA")
            nc.tensor.transpose(aT_ps[:], a_col[:], ident[:])
            aT_sb = smvp.tile([TB, 128], F32, tag="aT_sb")
            nc.vector.tensor_copy(aT_sb[:], aT_ps[:])
            a_row_dram = dramp.tile([TB, 128], F32)
            nc.sync.dma_start(a_row_dram[:], aT_sb[:])
            a_row_flat = a_row_dram[:].rearrange("a b -> (a b)").unsqueeze(0)
            cq_bc = constp.tile([128, T], F32)
            nc.sync.dma_start(cq_bc[:], a_row_flat.partition_broadcast(128))
            ck_bc = constp.tile([128, T], F32)
            nc.sync.dma_start(ck_bc[:], a_row_flat.partition_broadcast(128))
            # cq = a * ws_q * 45/(256*127) ; ck = a * ws_k / 127
            nc.vector.tensor_scalar(cq_bc[:], cq_bc[:], ws_bc[:, 0:1],
                                    45.0 / (256.0 * 127.0), AL.mult, AL.mult)
            nc.vector.tensor_scalar(ck_bc[:], ck_bc[:], ws_bc[:, 1:2], 1.0 / 127.0,
                                    AL.mult, AL.mult)
            cv_col = constp.tile([128, TB], F32)
            nc.vector.tensor_scalar(cv_col[:], a_col[:], ws_bc[:, 2:3],
                                    1.0 / (127.0 * 256.0), AL.mult, AL.mult)

            # ======== ternarize helper ========
            def ternarize_chunk(w_in, r0, cols, wi):
                wchunk = wloadp.tile([128, cols], F32, tag="wchunk")
                nc.sync.dma_start(wchunk[:], w_in[r0:r0 + 128, :])
                wsign = wternp.tile([128, cols], BF16, tag="wsign", bufs=1)
                nc.scalar.activation(wsign[:], wchunk[:], AF.Sign)
                nc.scalar.activation(wchunk[:], wchunk[:], AF.Abs)
                wmask = wternp.tile([128, cols], BF16, tag="wmask", bufs=1)
                nc.vector.tensor_scalar(wmask[:], wchunk[:], thr_bc[:, wi:wi + 1],
                                        None, AL.is_ge)
                wt = wternp.tile([128, cols], BF16, tag="wt")
                nc.vector.tensor_tensor(wt[:], wmask[:], wsign[:], AL.mult)
                return wt

            # ======== WoT: ternarize + transpose + AllGather (early) ========
            woT_loc_dram = dramp.tile([CL, C], BF16)
            for oc in range(CCH):
                wt = ternarize_chunk(wo_in, oc * 128, CL, 3)
                woT_sb = wtp.tile([128, CLH, 128], BF16, tag="woT_sb")
                for cl in range(CLH):
                    nc.sync.dma_start_transpose(
                        woT_sb[:, cl, :], wt[:, cl * 128:(cl + 1) * 128])
                for cl in range(CLH):
                    nc.sync.dma_start(
                        woT_loc_dram[cl * 128:(cl + 1) * 128,
                                     oc * 128:(oc + 1) * 128],
                        woT_sb[:, cl, :])
            woT_full_dram = dramp.tile([C, C], BF16)
            nc.gpsimd.collective_compute(
                "AllGather", AL.bypass, replica_groups=RG2,
                ins=[woT_loc_dram.opt()], outs=[woT_full_dram.opt()])

            # ======== per-head: QKV projection + attention ========
            xqo_strips = [oprojp.tile([128, CL], BF16, tag=f"xqo{tb}", name=f"xqo{tb}")
                          for tb in range(TB)]
            for h in range(HL):
                qkv_t = {}
                for wi, (w_in, nm) in enumerate([(wq_in, "q"), (wk_in, "k"),
                                                 (wv_in, "v")]):
                    wt = ternarize_chunk(w_in, h * 128, C, wi)
                    wtT = wtp.tile([128, CCH, 128], BF16, tag="wtT")
                    for cc in range(CCH):
                        nc.sync.dma_start_transpose(
                            wtT[:, cc, :], wt[:, cc * 128:(cc + 1) * 128])
                    ps_q = psp.tile([128, T], F32, tag="psbig")
                    for n0 in (0, 512):
                        for cc in range(CCH):
                            nc.tensor.matmul(
                                ps_q[:, n0:n0 + 512], wtT[:, cc, :],
                                xqT[cc][:, n0:n0 + 512],
                                start=(cc == 0), stop=(cc == CCH - 1))
                    ot = qkvp.tile([128, T], F32, tag=nm)
                    if nm == "q":
                        nc.vector.tensor_tensor(ot[:], ps_q[:], cq_bc[:], AL.mult)
                    elif nm == "k":
                        nc.vector.tensor_tensor(ot[:], ps_q[:], ck_bc[:], AL.mult)
                    else:
                        nc.scalar.activation(ot[:], ps_q[:], AF.Copy)
                    qkv_t[nm] = ot
                qT_h, kT_h, vT_h = qkv_t["q"], qkv_t["k"], qkv_t["v"]

                # v tiles [keys,128 x d,128] with cv/256 fold
                v_tiles = []
                for tb in range(TB):
                    ps_v = psAp.tile([128, 128], F32, tag="psA")
                    nc.tensor.transpose(ps_v[:], vT_h[:, tb * 128:(tb + 1) * 128],
                                        ident[:])
                    vt_ = attp.tile([128, 128], F32, tag=f"v{tb}")
                    nc.vector.tensor_scalar(vt_[:], ps_v[:], cv_col[:, tb:tb + 1],
                                            None, AL.mult)
                    v_tiles.append(vt_)

                for qb in range(TB):
                    L = (qb + 1) * 128
                    ps_s = psp.tile([128, T], F32, tag="psbig")
                    for n0 in range(0, L, 512):
                        nn = min(512, L - n0)
                        nc.tensor.matmul(ps_s[:, n0:n0 + nn],
                                         qT_h[:, qb * 128:(qb + 1) * 128],
                                         kT_h[:, n0:n0 + nn],
                                         start=True, stop=True)
                    yp = smp.tile([128, T], F32, tag="f1")
                    nc.scalar.activation(yp[:, :L], ps_s[:, :L], AF.Copy, bias=-0.5)
                    t_ = smp.tile([128, T], F32, tag="f2")
                    nc.vector.tensor_scalar(t_[:, :L], yp[:, :L], MAGIC, -MAGIC,
                                            AL.add, AL.add)
                    # mask upper triangle of the diagonal block
                    nc.vector.tensor_tensor(t_[:, qb * 128:L], t_[:, qb * 128:L],
                                            minmask[:], AL.min)
                    m_ = smvp.tile([128, 1], F32, tag="m_")
                    nc.vector.tensor_reduce(m_[:], t_[:, :L], axis=AX.X, op=AL.max)
                    m24 = smvp.tile([128, 1], F32, tag="m24")
                    nc.vector.tensor_scalar(m24[:], m_[:], -24.0, None, AL.add)
                    b2 = smvp.tile([128, 1], F32, tag="b2")
                    nc.vector.tensor_scalar(b2[:], m_[:], -10.0, 80.0, AL.mult, AL.add)
                    b3 = smvp.tile([128, 1], F32, tag="b3")
                    nc.vector.tensor_scalar(b3[:], m_[:], -53.0, 159.0, AL.mult, AL.add)
                    m8 = smvp.tile([128, 1], F32, tag="m8")
                    nc.vector.tensor_scalar(m8[:], m_[:], -8.0, None, AL.add)
                    r1 = smp.tile([128, T], BF16, tag="r1")
                    nc.vector.tensor_scalar(r1[:, :L], t_[:, :L], m24[:], 0.0,
                                            AL.subtract, AL.max)
                    r2 = smp.tile([128, T], BF16, tag="r2")
                    nc.scalar.activation(r2[:, :L], t_[:, :L], AF.Relu,
                                         bias=b2[:], scale=10.0)
                    r3 = smp.tile([128, T], BF16, tag="r3")
                    nc.scalar.activation(r3[:, :L], t_[:, :L], AF.Relu,
                                         bias=b3[:], scale=53.0)
                    m7 = smp.tile([128, T], BF16, tag="m7")
                    nc.vector.tensor_scalar(m7[:, :L], t_[:, :L], m8[:], -7.0,
                                            AL.is_ge, AL.mult)
                    nc.vector.tensor_tensor(r1[:, :L], r1[:, :L], r2[:, :L], AL.add)
                    nc.vector.tensor_tensor(r3[:, :L], r3[:, :L], m7[:, :L], AL.add)
                    e_ = r1
                    den = smvp.tile([128, 1], F32, tag="den")
                    nc.vector.scalar_tensor_tensor(e_[:, :L], r1[:, :L], 1.0,
                                                   r3[:, :L], AL.mult, AL.add,
                                                   accum_out=den[:])
                    rv = smvp.tile([128, 1], F32, tag="rv")
                    nc.vector.reciprocal(rv[:], den[:])
                    nc.vector.tensor_scalar(rv[:], rv[:], 255.0, None, AL.mult)
                    p1 = smp.tile([128, T], F32, tag="f1")
                    nc.vector.tensor_scalar(p1[:, :L], e_[:, :L], rv[:], MAGIC,
                                            AL.mult, AL.add)
                    probs = smp.tile([128, T], F32, tag="f2")
                    nc.vector.tensor_scalar(probs[:, :L], p1[:, :L], -MAGIC, None,
                                            AL.add)
                    # pv: transpose probs blocks (batched into 512-wide psum), MMs
                    ps_o = psBp.tile([128, 128], F32, tag="psB")
                    for g0 in range(0, qb + 1, 4):
                        gn = min(4, qb + 1 - g0)
                        ps_pt = psAp.tile([128, 512], F32, tag="psA")
                        for j in range(gn):
                            kb = g0 + j
                            nc.tensor.transpose(
                                ps_pt[:, j * 128:(j + 1) * 128],
                                probs[:, kb * 128:(kb + 1) * 128], ident[:])
                        pT = smp.tile([128, 512], F32, tag="pT")
                        if (g0 // 4) % 2 == 0:
                            nc.vector.tensor_copy(pT[:, :gn * 128],
                                                  ps_pt[:, :gn * 128])
                        else:
                            nc.scalar.activation(pT[:, :gn * 128],
                                                 ps_pt[:, :gn * 128], AF.Copy)
                        for j in range(gn):
                            kb = g0 + j
                            nc.tensor.matmul(ps_o[:], pT[:, j * 128:(j + 1) * 128],
                                             v_tiles[kb][:],
                                             start=(kb == 0), stop=(kb == qb))
                    o1 = smp.tile([128, 128], F32, tag="o1")
                    nc.scalar.activation(o1[:], ps_o[:], AF.Copy, bias=-0.5)
                    o2 = smp.tile([128, 128], F32, tag="o2")
                    nc.vector.tensor_scalar(o2[:], o1[:], MAGIC, -MAGIC,
                                            AL.add, AL.add)
                    nc.vector.tensor_scalar(
                        xqo_strips[qb][:, h * 128:(h + 1) * 128], o2[:],
                        127.0, -128.0, AL.min, AL.max)

            # ======== attn-out absmax exchange ========
            ao_col = constp.tile([128, TB], F32)
            for tb in range(TB):
                nc.vector.tensor_reduce(ao_col[:, tb:tb + 1], xqo_strips[tb][:],
                                        axis=AX.X, op=AL.max,
                                        apply_absolute_value=True)
            ao_src = dramp.tile([128, TB], F32)
            ao_dst = dramp.tile([128, TB], F32)
            nc.sync.dma_start(ao_src[:], ao_col[:])
            nc.gpsimd.collective_compute(
                "AllReduce", AL.max, replica_groups=RG2,
                ins=[ao_src.opt()], outs=[ao_dst.opt()])
            ao_all = constp.tile([128, TB], F32)
            nc.sync.dma_start(ao_all[:], ao_dst[:])
            nc.vector.tensor_scalar(ao_all[:], ao_all[:], 1e-5, None, AL.max)
            so_col = constp.tile([128, TB], F32)
            nc.vector.reciprocal(so_col[:], ao_all[:])
            nc.vector.tensor_scalar(so_col[:], so_col[:], 127.0, None, AL.mult)
            co_col = constp.tile([128, TB], F32)
            nc.vector.tensor_scalar(co_col[:], ao_all[:], ws_bc[:, 3:4], 1.0 / 127.0,
                                    AL.mult, AL.mult)
            co_dram = dramp.tile([128, TB], F32)
            nc.sync.dma_start(co_dram[:], co_col[:])

            # ======== xq_o + transpose + AllGather ========
            xqoT_loc_dram = dramp.tile([CL, T], BF16)
            for tb in range(TB):
                q1 = smp.tile([128, CL], F32, tag="f1")
                nc.vector.tensor_scalar(q1[:], xqo_strips[tb][:],
                                        so_col[:, tb:tb + 1], MAGIC,
                                        AL.mult, AL.add)
                q2 = smp.tile([128, CL], BF16, tag="q2")
                nc.vector.tensor_scalar(q2[:], q1[:], -MAGIC, None, AL.add)
                q2T = wtp.tile([128, CLH, 128], BF16, tag="q2T")
                for cl in range(CLH):
                    nc.sync.dma_start_transpose(
                        q2T[:, cl, :], q2[:, cl * 128:(cl + 1) * 128])
                for cl in range(CLH):
                    nc.sync.dma_start(
                        xqoT_loc_dram[cl * 128:(cl + 1) * 128,
                                      tb * 128:(tb + 1) * 128],
                        q2T[:, cl, :])
            xqoT_full_dram = dramp.tile([C, T], BF16)
            nc.gpsimd.collective_compute(
                "AllGather", AL.bypass, replica_groups=RG2,
                ins=[xqoT_loc_dram.opt()], outs=[xqoT_full_dram.opt()])

            # ======== O projection on own token half ========
            _cstack.close()
            opool = _cstack.enter_context(tc.tile_pool(name="ophase", bufs=2))
            pid = nc.partition_id()
            tok0 = (pid % 2) * (T // 2)
            co_own = opool.tile([128, 4], F32)
            nc.sync.dma_start(co_own[:], co_dram[:, ds((pid % 2) * 4, 4)])
            xoT = []
            for tb in range(4):
                xt_ = opool.tile([128, CCH, 128], BF16, tag=f"xoT{tb}", name=f"xoT{tb}")
                for cc in range(CCH):
                    nc.sync.dma_start(
                        xt_[:, cc, :],
                        xqoT_full_dram[cc * 128:(cc + 1) * 128,
                                       ds(tok0 + tb * 128, 128)])
                xoT.append(xt_)
            for og in range(4):
                for tb in range(4):
                    ps_y = psBp.tile([128, 512], F32, tag="psB")
                    for cc in range(CCH):
                        wslice = opool.tile([128, 512], BF16, tag="wslice")
                        nc.sync.dma_start(
                            wslice[:],
                            woT_full_dram[cc * 128:(cc + 1) * 128,
                                          og * 512:(og + 1) * 512])
                        nc.tensor.matmul(ps_y[:], xoT[tb][:, cc, :], wslice[:],
                                         start=(cc == 0), stop=(cc == CCH - 1))
                    y_sb = opool.tile([128, 512], F32, tag="y_sb")
                    nc.vector.tensor_scalar(y_sb[:], ps_y[:], co_own[:, tb:tb + 1],
                                            None, AL.mult)
                    nc.sync.dma_start(
                        y_out[tb * 128:(tb + 1) * 128, og * 512:(og + 1) * 512],
                        y_sb[:])
            _cstack.close()

    nc.finalize()
    return nc


_NC_CACHE = None


def kernel(x, Wq, Wk, Wv, Wo):
    global _NC_CACHE
    if _NC_CACHE is None:
        _NC_CACHE = build_nc()
    nc = _NC_CACHE

    x = np.ascontiguousarray(np.asarray(x, np.float32))
    Wq = np.ascontiguousarray(np.asarray(Wq, np.float32))
    Wk = np.ascontiguousarray(np.asarray(Wk, np.float32))
    Wv = np.ascontiguousarray(np.asarray(Wv, np.float32))
    Wo = np.ascontiguousarray(np.asarray(Wo, np.float32))

    in_maps = []
    for c in range(8):
        b, g = c // 2, c % 2
        in_maps.append({
            "x": np.ascontiguousarray(x[b]),
            "wq": np.ascontiguousarray(Wq[g * CL:(g + 1) * CL, :]),
            "wk": np.ascontiguousarray(Wk[g * CL:(g + 1) * CL, :]),
            "wv": np.ascontiguousarray(Wv[g * CL:(g + 1) * CL, :]),
            "wo": np.ascontiguousarray(Wo[:, g * CL:(g + 1) * CL]),
        })
    res = run_bass_kernel_spmd(nc, in_maps, core_ids=list(range(8)))
    y = np.empty((B, T, C), np.float32)
    for c in range(8):
        b, g = c // 2, c % 2
        y[b, g * (T // 2):(g + 1) * (T // 2), :] = res.results[c]["y"]
    return y
